# revision 12
# baseline (speedup 1.0000x reference)
"""PointHop octant-binning kernel for TRN2 (8 NeuronCores, B-sharded).

Per group g (G = B*N groups, K = 64 neighbors): std (ddof=1) of x,y,z;
center; per-octant mean of (x,y,z) with empty bins 0. Output [B, N, 30].

Groups-on-partitions layout: slab = 1024 groups = 128 partitions x 8
groups. Octant sums via subset-sum lattice T(s) = sum_k (prod m_d) v_c
computed with fused scalar_tensor_tensor (is_gt 0, mult), segmented
tensor_reduce, then a 3-round in-place butterfly (Moebius inversion).
Counts use the same lattice on the masks.
"""

import os
from contextlib import ExitStack

import numpy as np

# The Bass runner reaches the NeuronCores through the jax "axon" PJRT
# platform; a cpu-pinned JAX_PLATFORMS would hide them.
if "axon" not in os.environ.get("JAX_PLATFORMS", "axon"):
    os.environ.pop("JAX_PLATFORMS", None)

import concourse.bass as bass
import concourse.bacc as bacc
import concourse.tile as tile
from concourse import mybir
from concourse.bass_utils import run_bass_kernel_spmd

B, C, N, K = 32, 3, 8192, 64
NCORES = 8
BL = B // NCORES          # 4 batches per core
PART = 128
TG = 8                    # groups per partition per slab
SLAB = PART * TG          # 1024 groups per slab
NSLAB = BL * N // SLAB    # 32 slabs per core
FOUT = 30

AL = mybir.AluOpType
AF = mybir.ActivationFunctionType
F32 = mybir.dt.float32


def _build_kernel(nc: bass.Bass, repeat: int = 1):
    gx = nc.dram_tensor("gx", [BL, C, N, K], F32, kind="ExternalInput")
    nx = nc.dram_tensor("nx", [BL, N, C], F32, kind="ExternalInput")
    out = nc.dram_tensor("out", [BL, N, FOUT], F32, kind="ExternalOutput")

    with tile.TileContext(nc) as tc, ExitStack() as ctx:
        vpool = ctx.enter_context(tc.tile_pool(name="v", bufs=2))
        ppool = ctx.enter_context(tc.tile_pool(name="p", bufs=2))
        spool = ctx.enter_context(tc.tile_pool(name="s", bufs=2))
        opool = ctx.enter_context(tc.tile_pool(name="o", bufs=2))

        for slab in [s for _ in range(repeat) for s in range(NSLAB)]:
            b, s = divmod(slab, N // SLAB)
            n0 = s * SLAB

            # ---- loads ----
            V = vpool.tile([PART, C * TG * K], F32)
            nc.sync.dma_start(
                out=V[:].rearrange("p (c t k) -> p c t k", c=C, t=TG),
                in_=gx[b, :, n0:n0 + SLAB, :].rearrange(
                    "c (p t) k -> p c t k", p=PART, t=TG))
            CIN = vpool.tile([PART, TG * C], F32)
            nc.sync.dma_start(
                out=CIN[:].rearrange("p (t c) -> p t c", t=TG),
                in_=nx[b, n0:n0 + SLAB, :].rearrange(
                    "(p t) c -> p t c", p=PART, t=TG))

            def vc(c):  # coord slice [128, 512] with [t, k] layout
                return V[:, c * TG * K:(c + 1) * TG * K]

            VX, VY, VZ = vc(0), vc(1), vc(2)

            # ---- products: P_c sections u=1..7 (u = 4bx+2by+bz) ----
            P = [ppool.tile([PART, 7 * TG * K], F32, name=f"P{c}")
                 for c in range(3)]

            def sec(c, u):
                return P[c][:, (u - 1) * TG * K:u * TG * K]

            stt = nc.vector.scalar_tensor_tensor
            sttg = nc.vector.scalar_tensor_tensor
            relu = lambda o, i: nc.scalar.activation(o, i, AF.Relu)

            # coord x (own bit u=4): u1=mz*x u2=my*x u3=my*(mz*x)
            #   u4=relu(x) u5=mz*x+ u6=my*x+ u7=my*(mz*x+)
            relu(sec(0, 4), VX)
            sttg(sec(0, 1), VZ, 0.0, VX, AL.is_gt, AL.mult)
            sttg(sec(0, 2), VY, 0.0, VX, AL.is_gt, AL.mult)
            sttg(sec(0, 3), VY, 0.0, sec(0, 1), AL.is_gt, AL.mult)
            stt(sec(0, 5), VZ, 0.0, sec(0, 4), AL.is_gt, AL.mult)
            stt(sec(0, 6), VY, 0.0, sec(0, 4), AL.is_gt, AL.mult)
            stt(sec(0, 7), VY, 0.0, sec(0, 5), AL.is_gt, AL.mult)
            # coord y (own bit u=2)
            relu(sec(1, 2), VY)
            sttg(sec(1, 1), VZ, 0.0, VY, AL.is_gt, AL.mult)
            sttg(sec(1, 4), VX, 0.0, VY, AL.is_gt, AL.mult)
            sttg(sec(1, 5), VX, 0.0, sec(1, 1), AL.is_gt, AL.mult)
            stt(sec(1, 3), VZ, 0.0, sec(1, 2), AL.is_gt, AL.mult)
            stt(sec(1, 6), VX, 0.0, sec(1, 2), AL.is_gt, AL.mult)
            stt(sec(1, 7), VX, 0.0, sec(1, 3), AL.is_gt, AL.mult)
            # coord z (own bit u=1)
            relu(sec(2, 1), VZ)
            sttg(sec(2, 2), VY, 0.0, VZ, AL.is_gt, AL.mult)
            sttg(sec(2, 4), VX, 0.0, VZ, AL.is_gt, AL.mult)
            sttg(sec(2, 6), VX, 0.0, sec(2, 2), AL.is_gt, AL.mult)
            stt(sec(2, 3), VY, 0.0, sec(2, 1), AL.is_gt, AL.mult)
            stt(sec(2, 5), VX, 0.0, sec(2, 1), AL.is_gt, AL.mult)
            stt(sec(2, 7), VX, 0.0, sec(2, 3), AL.is_gt, AL.mult)

            # ---- count products: masks, sections u=1..7 ----
            CP = ppool.tile([PART, 7 * TG * K], F32)

            def csec(u):
                return CP[:, (u - 1) * TG * K:u * TG * K]

            ts = nc.vector.tensor_scalar
            ts(csec(1), VZ, 0.0, None, AL.is_gt)
            ts(csec(2), VY, 0.0, None, AL.is_gt)
            ts(csec(4), VX, 0.0, None, AL.is_gt)
            nc.gpsimd.tensor_tensor(csec(3), csec(1), csec(2), AL.mult)
            nc.gpsimd.tensor_tensor(csec(5), csec(1), csec(4), AL.mult)
            nc.gpsimd.tensor_tensor(csec(6), csec(2), csec(4), AL.mult)
            nc.gpsimd.tensor_tensor(csec(7), csec(3), csec(4), AL.mult)

            # squares for std
            SQ = ppool.tile([PART, C * TG * K], F32)
            nc.scalar.activation(SQ[:], V[:], AF.Square)

            # ---- reduces ----
            # ST layout [c, u, t]; CT [u, t]; SS/Q/D [c, t]
            ST = spool.tile([PART, C * 8 * TG], F32)
            CT = spool.tile([PART, 8 * TG], F32)
            SS = spool.tile([PART, C * TG], F32)
            red = nc.vector.tensor_reduce
            X = mybir.AxisListType.X
            st4 = ST[:].rearrange("p (c u t) -> p c u t", c=C, u=8)
            for c in range(3):
                red(ST[:, c * 64 + TG:(c + 1) * 64],
                    P[c][:].rearrange("p (u t k) -> p u t k", u=7, t=TG),
                    X, AL.add)
            red(st4[:, :, 0:1, :],
                V[:].rearrange("p (c t k) -> p (c t) k", c=C, t=TG),
                X, AL.add)
            red(CT[:, TG:],
                CP[:].rearrange("p (u t k) -> p u t k", u=7, t=TG),
                X, AL.add)
            nc.vector.memset(CT[:, 0:TG], float(K))
            red(SS[:], SQ[:].rearrange("p (c t k) -> p (c t) k", c=C, t=TG),
                X, AL.add)

            # (Sum_k v)^2 / 64, from u0 column before the butterfly
            Q = spool.tile([PART, C * TG], F32)
            nc.scalar.activation(Q[:], st4[:, :, 0:1, :], AF.Square,
                                 0.0, 1.0 / 8.0)

            # ---- butterfly: subset sums -> exact octant sums (in place)
            sub = nc.vector.tensor_tensor
            sub(st4[:, :, 0:4, :], st4[:, :, 0:4, :], st4[:, :, 4:8, :],
                AL.subtract)
            ct3 = CT[:].rearrange("p (u t) -> p u t", u=8)
            sub(ct3[:, 0:4, :], ct3[:, 0:4, :], ct3[:, 4:8, :], AL.subtract)
            # keep every compute AP at <= 4 total dims (HW limit):
            # merge (c, a) into one dim m for rounds 2 and 3
            st5 = ST[:].rearrange("p (m u t) -> p m u t", m=C * 2, u=4)
            sub(st5[:, :, 0:2, :], st5[:, :, 0:2, :],
                st5[:, :, 2:4, :], AL.subtract)
            ct4 = CT[:].rearrange("p (a u t) -> p a u t", a=2, u=4)
            sub(ct4[:, :, 0:2, :], ct4[:, :, 0:2, :], ct4[:, :, 2:4, :],
                AL.subtract)
            st6 = ST[:].rearrange("p (m u t) -> p m u t", m=C * 4, u=2)
            sub(st6[:, :, 0:1, :], st6[:, :, 0:1, :],
                st6[:, :, 1:2, :], AL.subtract)
            ct5 = CT[:].rearrange("p (a u t) -> p a u t", a=4, u=2)
            sub(ct5[:, :, 0:1, :], ct5[:, :, 0:1, :], ct5[:, :, 1:2, :],
                AL.subtract)

            # ---- epilogue ----
            CC = spool.tile([PART, 8 * TG], F32)
            RC = spool.tile([PART, 8 * TG], F32)
            nc.vector.tensor_scalar_max(CC[:], CT[:], 1.0)
            nc.vector.reciprocal(RC[:], CC[:])

            O = opool.tile([PART, TG * FOUT], F32)
            ov = O[:].rearrange("p (t f) -> p t f", t=TG)

            # means: S[c,u,t] * (1/count)[u,t] -> O[t, 6 + u*3 + c]
            mn = ov[:, :, 6:].rearrange("p t (u c) -> p c u t", u=8, c=C)
            rc3 = RC[:].rearrange("p (u t) -> p u t", u=8)
            for c in range(3):
                nc.gpsimd.tensor_tensor(mn[:, c], st4[:, c], rc3, AL.mult)

            # std = sqrt((SS - Q)/63) -> O[t, 0:3]
            D = spool.tile([PART, C * TG], F32)
            nc.vector.tensor_sub(D[:], SS[:], Q[:])
            nc.scalar.activation(
                ov[:, :, 0:3],
                D[:].rearrange("p (c t) -> p t c", c=C),
                AF.Sqrt, 0.0, 1.0 / 63.0)
            # center -> O[t, 3:6]
            nc.gpsimd.tensor_copy(
                ov[:, :, 3:6],
                CIN[:].rearrange("p (t c) -> p t c", t=TG))

            nc.sync.dma_start(
                out=out[b, n0:n0 + SLAB, :].rearrange(
                    "(p t) f -> p t f", p=PART, t=TG),
                in_=O[:].rearrange("p (t f) -> p t f", t=TG))


_CACHE: dict = {}


def _get_nc():
    if "nc" not in _CACHE:
        nc = bacc.Bacc("TRN2", target_bir_lowering=False, debug=False)
        _build_kernel(nc)
        nc.finalize()
        _CACHE["nc"] = nc
    return _CACHE["nc"]


def kernel(group_xyz: np.ndarray, new_xyz: np.ndarray) -> np.ndarray:
    nc = _get_nc()
    gx = np.ascontiguousarray(group_xyz, dtype=np.float32)
    nx = np.ascontiguousarray(new_xyz, dtype=np.float32)
    in_maps = [
        {"gx": gx[i * BL:(i + 1) * BL], "nx": nx[i * BL:(i + 1) * BL]}
        for i in range(NCORES)
    ]
    res = run_bass_kernel_spmd(nc, in_maps, list(range(NCORES)))
    return np.concatenate([res.results[i]["out"] for i in range(NCORES)],
                          axis=0)
